# revision 11
# baseline (speedup 1.0000x reference)
"""Trainium2 Bass kernel for nn_MoEExpertPool — expert x batch hybrid sharding.

Grid: 8 cores = 4 expert-groups x 2 batch-halves.  Core c handles experts
{3g, 3g+1, 3g+2} (g = c % 4) on batch half h = c // 4 (2048 rows).

Per (expert, mt) the weight strip is loaded once and streamed over 4 moving
tiles of 512 batch columns (weight-stationary), so per-core weight DMA is
75.5 MB/rep (vs 604 MB for pure batch-parallel) and LDWEIGHTS cost is
amortized 4x (layer1) / 2x (layer2).

The PoE combine runs on host: each core emits per-expert mu-contribution
(w_e * (mu_e) * prec_e ... stored as (pmu+bmu)*prec*wg) and prec_e = exp(-lv_e)
tiles; host sums the 12 experts per batch half in f64 and finalizes.
All matmul operands are bf16 (rel err ~5e-3, gate is 2e-2).
"""

import contextlib
import os
import sys

sys.path.insert(0, "/opt/trn_rl_repo")

import numpy as np

B, D, E, NG = 4096, 2048, 4, 3
NEXP = NG * E            # 12 experts
N_CORES = 8
BH = B // 2              # 2048 batch rows per core (one half)
EPC = 3                  # experts per core
MT = D // 128            # 16 output tiles
KT = D // 128            # 16 contraction tiles
BT = BH // 512           # 4 moving tiles of 512
EPS = 1e-8

W_BUFS = int(os.environ.get("V3_WBUFS", "6"))

_cache = {}


def _bcol(e, j, mt):
    return (e * 3 + j) * MT + mt


def _dedupe_ldweights(nc):
    """Remove InstLdweights that reload the weight tile already resident in
    the PE array (tile_legalize emits one per matmul even for consecutive
    same-weight matmuls, and walrus runs with ldw-opt disabled so they are
    never elided).  A deleted load's semaphore waits/updates merge into the
    next PE instruction (its paired matmul) so DMA ordering and buffer-reuse
    gating are preserved.  Keys are structural AP reprs; tracking resets per
    block so loop back-edges stay conservative."""
    removed = 0
    for f in nc.m.functions:
        for blk in f.blocks:
            insts = blk.instructions
            last_key = None
            to_del = []
            for idx, inst in enumerate(insts):
                if type(inst).__name__ == "InstLdweights":
                    key = repr(inst.ins[0])
                    if key == last_key:
                        to_del.append(idx)
                    else:
                        last_key = key
            for idx in reversed(to_del):
                inst = insts[idx]
                si = inst.sync_info
                if si is not None and (len(si.on_wait) or len(si.on_update)):
                    # merge into the next PE instruction
                    import concourse.mybir as mybir

                    for j in range(idx + 1, len(insts)):
                        nxt = insts[j]
                        if getattr(nxt, "engine", None) == mybir.EngineType.PE:
                            nsi = nxt.sync_info
                            if nsi is None:
                                nxt.sync_info = mybir.SyncInfo(
                                    on_wait=list(si.on_wait),
                                    on_update=list(si.on_update),
                                )
                            else:
                                nsi.on_wait = list(nsi.on_wait) + list(si.on_wait)
                                nsi.on_update = (
                                    list(nsi.on_update) + list(si.on_update)
                                )
                            break
                del insts[idx]
                removed += 1
    return removed


def _build_nc(reps=1):
    import concourse.mybir as mybir
    import concourse.tile as tile
    from concourse import bacc

    f32 = mybir.dt.float32
    mmdt = mybir.dt.bfloat16
    AF = mybir.ActivationFunctionType

    nc = bacc.Bacc("TRN2", target_bir_lowering=False)
    xT = nc.dram_tensor("xT", [D, BH], mmdt, kind="ExternalInput")
    # pre-tiled weight strips: [matrix, mt, p, kt, m], each strip contiguous
    W = nc.dram_tensor("W", [EPC * 3, MT, 128, KT, 128], mmdt, kind="ExternalInput")
    WG = nc.dram_tensor("WG", [EPC, BH], f32, kind="ExternalInput")
    BIAS = nc.dram_tensor("BIAS", [128, EPC * 3 * MT], f32, kind="ExternalInput")
    MUC = nc.dram_tensor("MUC", [EPC, D, BH], f32, kind="ExternalOutput")
    PRC = nc.dram_tensor("PRC", [EPC, D, BH], f32, kind="ExternalOutput")

    with tile.TileContext(nc) as tc:
        with (
            tc.tile_pool(name="xp", bufs=1) as xp,
            tc.tile_pool(name="hp", bufs=1) as hp,
            tc.tile_pool(name="wp", bufs=W_BUFS) as wp,
            tc.tile_pool(name="gp", bufs=2) as gp,
            tc.tile_pool(name="cp", bufs=1) as cp,
            tc.tile_pool(name="ew", bufs=4) as ew,
            # one 8-bank PSUM pool: [128, 4, 512] quad tiles, double-buffered.
            # Every matmul chain streams 4 moving tiles against one weight
            # tile (weight-stationary x4 in both layers).
            tc.tile_pool(name="psq", bufs=2, space="PSUM") as psq,
        ):
            xsb = xp.tile([128, KT, BH], mmdt)
            nc.sync.dma_start(xsb[:], xT[:, :].rearrange("(kt p) b -> p kt b", p=128))
            bias_sb = cp.tile([128, EPC * 3 * MT], f32)
            nc.sync.dma_start(bias_sb[:], BIAS[:, :])

            rep_loop = (
                tc.For_i(0, reps, 1) if reps > 1 else contextlib.nullcontext()
            )
            with rep_loop:
                for e in range(EPC):
                    wg_t = gp.tile([128, BH], f32, tag="wg")
                    nc.sync.dma_start(
                        wg_t[:], WG[e : e + 1, :].partition_broadcast(128)
                    )

                    h = hp.tile([128, KT, BH], mmdt, tag="h")
                    # layer 1: hT = relu(W1.T @ xT + b1), weight-stationary
                    # over 4 moving tiles per (mt, kt)
                    for mt in range(MT):
                        wst = wp.tile([128, KT, 128], mmdt, tag="wstrip")
                        nc.sync.dma_start(wst[:], W[3 * e, mt])
                        ps4 = psq.tile([128, BT, 512], f32, tag="quad")
                        for kt in range(KT):
                            for bt in range(BT):
                                nc.tensor.matmul(
                                    ps4[:, bt, :],
                                    wst[:, kt, :],
                                    xsb[:, kt, bt * 512 : (bt + 1) * 512],
                                    start=(kt == 0),
                                    stop=(kt == KT - 1),
                                )
                        for bt in range(BT):
                            nc.scalar.activation(
                                h[:, mt, bt * 512 : (bt + 1) * 512],
                                ps4[:, bt, :],
                                AF.Relu,
                                bias=bias_sb[:, _bcol(e, 0, mt) : _bcol(e, 0, mt) + 1],
                            )
                    # layer 2: mu/lv in bt-pairs (weights reused x2 per load,
                    # 4 psum banks: pmu2 + plv2)
                    for mt in range(MT):
                        wmu = wp.tile([128, KT, 128], mmdt, tag="wstrip")
                        nc.sync.dma_start(wmu[:], W[3 * e + 1, mt])
                        wlv = wp.tile([128, KT, 128], mmdt, tag="wstrip")
                        nc.sync.dma_start(wlv[:], W[3 * e + 2, mt])
                        # lv chain FIRST: its consumers (exp, precw) overlap
                        # with the mu chain; stt fires right after the mu
                        # chain stops -> the next quad allocation never waits.
                        plv4 = psq.tile([128, BT, 512], f32, tag="quad")
                        for kt in range(KT):
                            for bt in range(BT):
                                nc.tensor.matmul(
                                    plv4[:, bt, :],
                                    wlv[:, kt, :],
                                    h[:, kt, bt * 512 : (bt + 1) * 512],
                                    start=(kt == 0),
                                    stop=(kt == KT - 1),
                                )
                        precws = []
                        for bt in range(BT):
                            # prec = exp(-(plv + blv)); blv pre-negated
                            prec = ew.tile([128, 512], f32, tag="prec")
                            nc.scalar.activation(
                                prec[:], plv4[:, bt, :], AF.Exp, scale=-1.0,
                                bias=bias_sb[
                                    :, _bcol(e, 2, mt) : _bcol(e, 2, mt) + 1
                                ],
                            )
                            nc.sync.dma_start(
                                PRC[
                                    e,
                                    mt * 128 : (mt + 1) * 128,
                                    bt * 512 : (bt + 1) * 512,
                                ],
                                prec[:],
                            )
                            precw = ew.tile([128, 512], f32, tag="precw")
                            nc.vector.tensor_mul(
                                precw[:], prec[:],
                                wg_t[:, bt * 512 : (bt + 1) * 512],
                            )
                            precws.append(precw)
                        pmu4 = psq.tile([128, BT, 512], f32, tag="quad")
                        for kt in range(KT):
                            for bt in range(BT):
                                nc.tensor.matmul(
                                    pmu4[:, bt, :],
                                    wmu[:, kt, :],
                                    h[:, kt, bt * 512 : (bt + 1) * 512],
                                    start=(kt == 0),
                                    stop=(kt == KT - 1),
                                )
                        for bt in range(BT):
                            mu = ew.tile([128, 512], f32, tag="mu")
                            nc.vector.scalar_tensor_tensor(
                                mu[:], pmu4[:, bt, :],
                                bias_sb[:, _bcol(e, 1, mt) : _bcol(e, 1, mt) + 1],
                                precws[bt][:],
                                op0=mybir.AluOpType.add,
                                op1=mybir.AluOpType.mult,
                            )
                            nc.sync.dma_start(
                                MUC[
                                    e,
                                    mt * 128 : (mt + 1) * 128,
                                    bt * 512 : (bt + 1) * 512,
                                ],
                                mu[:],
                            )

    n = _dedupe_ldweights(nc)
    print(f"[kernel] deduped {n} redundant InstLdweights", flush=True)
    nc.compile()
    return nc


def _get_nc(reps=1):
    key = ("nc", reps)
    if key not in _cache:
        _cache[key] = _build_nc(reps)
    return _cache[key]


def _host_prep(inputs):
    import ml_dtypes

    bf16 = ml_dtypes.bfloat16
    x = np.asarray(inputs["x"], np.float32)
    mask = np.asarray(inputs["modality_mask"])
    xd = x.astype(np.float64)
    mask_mean = mask.astype(np.float64).mean(axis=1, keepdims=True)

    prefs = ["fs", "cb", "sp"]
    wgate = np.empty((NEXP, B), np.float32)
    for g, pref in enumerate(prefs):
        logits = xd @ np.asarray(inputs[f"{pref}_Wg"], np.float64) + np.asarray(
            inputs[f"{pref}_bg"], np.float64
        )
        logits -= logits.max(axis=1, keepdims=True)
        ex = np.exp(logits)
        w = ex / ex.sum(axis=1, keepdims=True)
        if pref == "cb":
            w = w * (1.0 - mask_mean)
        wgate[g * E : (g + 1) * E, :] = w.T.astype(np.float32)

    # strips pre-tiled [matrix, mt, p, kt, m]; biases packed per group of 3
    # experts (blv negated for the exp(-lv) activation)
    Wall = np.empty((NEXP, 3, MT, 128, KT, 128), bf16)
    ball = np.zeros((NEXP, 3, 128, MT), np.float32)
    for g, pref in enumerate(prefs):
        for e in range(E):
            ge = g * E + e
            for j, nm in enumerate(["W1", "Wmu", "Wlv"]):
                w = np.asarray(inputs[f"{pref}_{nm}"][e])
                Wall[ge, j] = (
                    w.reshape(KT, 128, MT, 128).transpose(2, 1, 0, 3).astype(bf16)
                )
            for j, nm in enumerate(["b1", "bmu", "blv"]):
                vec = np.asarray(inputs[f"{pref}_{nm}"][e], np.float32)
                if nm == "blv":
                    vec = -vec
                ball[ge, j] = vec.reshape(MT, 128).T

    xt = np.ascontiguousarray(x.T.astype(bf16))  # [D, B]
    in_maps = []
    for c in range(N_CORES):
        g, half = c % 4, c // 4
        exps = [3 * g, 3 * g + 1, 3 * g + 2]
        bias_arr = np.zeros((128, EPC * 3 * MT), np.float32)
        for ei, ge in enumerate(exps):
            for j in range(3):
                bias_arr[:, (ei * 3 + j) * MT : (ei * 3 + j + 1) * MT] = ball[ge, j]
        in_maps.append(
            {
                "xT": np.ascontiguousarray(xt[:, half * BH : (half + 1) * BH]),
                "W": np.ascontiguousarray(
                    Wall[exps].reshape(EPC * 3, MT, 128, KT, 128)
                ),
                "WG": np.ascontiguousarray(
                    wgate[exps, half * BH : (half + 1) * BH]
                ),
                "BIAS": bias_arr,
            }
        )
    return in_maps


def _finalize(results):
    mu_fused = np.empty((B, D), np.float32)
    lv_fused = np.empty((B, D), np.float32)
    for half in range(2):
        S1 = np.zeros((D, BH), np.float64)
        S2 = np.zeros((D, BH), np.float64)
        for g in range(4):
            r = results[half * 4 + g]
            S1 += r["MUC"].astype(np.float64).sum(axis=0)
            S2 += r["PRC"].astype(np.float64).sum(axis=0)
        sl = slice(half * BH, (half + 1) * BH)
        mu_fused[sl] = (S1 / S2).T.astype(np.float32)
        lv_fused[sl] = np.log(1.0 / S2 + EPS).T.astype(np.float32)
    return mu_fused, lv_fused


def kernel(run_kwargs=None, **inputs):
    from concourse.bass_utils import run_bass_kernel_spmd

    nc = _get_nc()
    in_maps = _host_prep(inputs)
    res = run_bass_kernel_spmd(
        nc, in_maps, core_ids=list(range(N_CORES)), **(run_kwargs or {})
    )
    _cache["last_result"] = res
    return _finalize(res.results)
